# revision 19
# baseline (speedup 1.0000x reference)
"""Trainium2 Bass kernel for nn_Attend (l2-distance attention with zero-kv).

Reference computation (per b,h):
    k' = [0; k], v' = [0; v]                       (prepend zero kv)
    scores[i,j] = (2 q_i.k'_j - |q_i|^2 - |k'_j|^2) * (D+2)^-0.5
    causal: j <= i+1 in padded index space
    out = softmax(scores) @ v'

Kernel algebra: softmax is invariant to the per-row constant -scale*|q_i|^2,
so with p~[i,j] = exp(2*scale*q_i.k_j) * exp(-scale*|k_j|^2) and the zero
column contributing exp(0)=1 to the denominator only:
    out_i = (sum_j p~ v_j) / (1 + sum_j p~)

Layout: scores are computed TRANSPOSED ([kv, q]) so P^T is directly the
moving operand of the PV matmul (no P transposes).  exp(-scale*|k|^2) is
folded into the PV stationary operand [V | 1] per kv partition; 2*scale is
folded into the exp activation's free affine scale.

The PE streams the moving operand at half rate when the contraction dim is
<= 64, so heads are processed in PAIRS with K=128: one gpsimd cast-DMA
packs both heads' q (and k) fp32->bf16 into [n, 128] DRAM staging, one
HWDGE DMA-transpose lands qT2/kT2 [128, n] in SBUF (head A d-rows in
partitions 0:64, head B in 64:128).  qT2 is the shared moving operand;
the per-head score matmuls use stationary operands kTA/kTB whose
other-head partition rows are ZERO (zeros in the stationary are free:
matmul time is set by the moving columns).

Sharding: 32 (b,h) pairs -> 4 heads per core, 8 cores, pure data parallel.
"""

import sys

for _p in ("/opt/trn_rl_repo", "/root/.axon_site"):
    if _p not in sys.path:
        sys.path.insert(0, _p)

import numpy as np

B, H, N, D = 2, 16, 2048, 64
NCORES = 8
HPC = (B * H) // NCORES          # heads per core = 4
SCALE = float((D + 2) ** -0.5)   # augmented head dim, matches reference
NB = N // 128                    # kv blocks of 128 = 16
NQT = N // 512                   # q tiles of 512 = 4

_BUILT = {}


def _build(qk_dt="bfloat16", pv_dt="bfloat16", hpc=HPC, n=N):
    """Build + finalize the SPMD Bass program (one core's view)."""
    assert qk_dt == "bfloat16" and pv_dt == "bfloat16", "v3 builder is bf16-only"
    assert hpc % 2 == 0, "heads processed in pairs"
    NB = n // 128
    NQT = n // 512
    import concourse.mybir as mybir
    import concourse.tile as tile
    from concourse import bacc
    from concourse.masks import make_identity

    f32 = mybir.dt.float32
    bf16 = mybir.dt.bfloat16
    Exp = mybir.ActivationFunctionType.Exp
    add = mybir.AluOpType.add

    nc = bacc.Bacc(
        "TRN2", target_bir_lowering=False, debug=False, num_swdge_queues=4
    )
    q_p = nc.declare_dram_parameter("q", [hpc, n, D], f32, isOutput=False)
    k_p = nc.declare_dram_parameter("k", [hpc, n, D], f32, isOutput=False)
    v_p = nc.declare_dram_parameter("v", [hpc, n, D], f32, isOutput=False)
    m_p = nc.declare_dram_parameter("masks", [128, 4 * 1024], bf16, isOutput=False)
    o_p = nc.declare_dram_parameter("out", [hpc, n, D], f32, isOutput=True)

    npairs = hpc // 2

    with tile.TileContext(nc) as tc:
        with (
            tc.tile_pool(name="stg", bufs=2, space="DRAM") as stgp,
            tc.tile_pool(name="const", bufs=1) as constp,
            tc.tile_pool(name="io", bufs=2) as iop,
            tc.tile_pool(name="kqt", bufs=2) as kqtp,
            tc.tile_pool(name="pt", bufs=6) as ptp,
            tc.tile_pool(name="fin", bufs=4) as finp,
            tc.tile_pool(name="vop", bufs=4) as vop,
            tc.tile_pool(name="ps_s", bufs=3, space="PSUM") as ps_s,
            tc.tile_pool(name="ps_af", bufs=2, space="PSUM") as ps_af,
        ):
            ident = constp.tile([128, 128], f32, tag="ident")
            make_identity(nc, ident[:])
            maskt = constp.tile([128, 4 * 1024], bf16, tag="maskt")
            nc.scalar.dma_start(out=maskt[:], in_=m_p[:])

            # kTA/kTB slots for every pair; zero halves memset ONCE up-front
            # (copies only ever write the data half, so zeros stay clean).
            kTAs, kTBs = [], []
            for pair in range(npairs):
                kTA = kqtp.tile([128, n], bf16, tag="kTA", name=f"kTA_{pair}")
                kTB = kqtp.tile([128, n], bf16, tag="kTB", name=f"kTB_{pair}")
                nc.vector.memset(kTA[64:128, :], 0.0)
                nc.vector.memset(kTB[0:64, :], 0.0)
                kTAs.append(kTA)
                kTBs.append(kTB)

            # ---- staging for ALL pairs up-front ----------------------
            qT2s = []
            for pair in range(npairs):
                hA, hB = 2 * pair, 2 * pair + 1
                stq = stgp.tile([n, 128], bf16, tag="stq")
                stk = stgp.tile([n, 128], bf16, tag="stk")
                nc.gpsimd.dma_start(out=stq[:, 0:64], in_=q_p[hA])
                nc.gpsimd.dma_start(out=stq[:, 64:128], in_=q_p[hB])
                nc.gpsimd.dma_start(out=stk[:, 0:64], in_=k_p[hA])
                nc.gpsimd.dma_start(out=stk[:, 64:128], in_=k_p[hB])
                qT2 = kqtp.tile([128, n], bf16, tag="qT2", name=f"qT2_{pair}")
                kT2 = kqtp.tile([128, n], bf16, tag="kT2", name=f"kT2_{pair}")
                nc.sync.dma_start(out=qT2[:], in_=stq[:], transpose=True)
                nc.sync.dma_start(out=kT2[:], in_=stk[:], transpose=True)
                nc.vector.tensor_copy(kTAs[pair][0:64, :], kT2[0:64, :])
                nc.vector.tensor_copy(kTBs[pair][64:128, :], kT2[64:128, :])
                qT2s.append(qT2)

            for pair in range(npairs):
                hA, hB = 2 * pair, 2 * pair + 1
                qT2 = qT2s[pair]
                kTA = kTAs[pair]
                kTB = kTBs[pair]

                # ---- per-head: [V*ek | ek] --------------------------
                vos = []
                for h in (hA, hB):
                    kn = iop.tile([128, NB, 64], f32, tag="kn")
                    vn = iop.tile([128, NB, 64], f32, tag="vn")
                    vo = vop.tile([128, NB, 65], bf16, tag="vo")
                    nc.scalar.dma_start(
                        out=kn[:], in_=k_p[h].rearrange("(b p) d -> p b d", p=128)
                    )
                    nc.scalar.dma_start(
                        out=vn[:], in_=v_p[h].rearrange("(b p) d -> p b d", p=128)
                    )
                    scr2 = iop.tile([128, NB, 64], f32, tag="scr2")
                    ksqs = iop.tile([128, NB], f32, tag="ksqs")
                    nc.vector.tensor_mul(scr2[:], kn[:], kn[:])
                    nc.vector.tensor_reduce(
                        ksqs[:], scr2[:], mybir.AxisListType.X, add
                    )
                    ek = iop.tile([128, NB], f32, tag="ek")
                    nc.scalar.activation(ek[:], ksqs[:], Exp, scale=-SCALE)
                    for b in range(NB):
                        nc.vector.tensor_scalar_mul(
                            vo[:, b, 0:64], vn[:, b, :], ek[:, b : b + 1]
                        )
                    nc.vector.tensor_copy(vo[:, :, 64:65], ek[:])
                    vos.append(vo)
                voA, voB = vos

                # ---- main flash loop (both heads per block) ----------
                pending = []  # deferred finalize work: (h, t, acc_sb)

                def emit_finalize():
                    h, t, acc_sb = pending.pop(0)
                    ptr4 = ps_s.tile(
                        [128, 4, 65], f32, tag="sp", name=f"ptr4_{h}_{t}"
                    )
                    for s in range(4):
                        nc.tensor.matmul(
                            ptr4[:, s, :],
                            acc_sb[:, 128 * s : 128 * (s + 1)],
                            ident[0:65, 0:65],
                            is_transpose=True,
                            start=(s == 0),
                            stop=(s == 3),
                        )
                    outt = finp.tile([128, 4, 64], f32, tag="outt")
                    dr = finp.tile([128, 8], f32, tag="dr")
                    nc.vector.tensor_scalar_add(dr[:, 0:4], ptr4[:, :, 64], 1.0)
                    nc.vector.reciprocal(dr[:, 4:8], dr[:, 0:4])
                    for s in range(4):
                        nc.vector.tensor_scalar_mul(
                            outt[:, s, :],
                            ptr4[:, s, 0:64],
                            dr[:, 4 + s : 5 + s],
                        )
                    nc.scalar.dma_start(
                        out=o_p[h].rearrange("(s p) d -> p s d", p=128)[
                            :, 4 * t : 4 * (t + 1), :
                        ],
                        in_=outt[:],
                    )

                for t in range(NQT):
                    nblk = 4 * (t + 1)
                    accA = ps_af.tile(
                        [65, 512], f32, tag="af", name=f"accA_{pair}_{t}"
                    )
                    accB = ps_af.tile(
                        [65, 512], f32, tag="af", name=f"accB_{pair}_{t}"
                    )
                    qs = qT2[:, 512 * t : 512 * (t + 1)]
                    for j in range(nblk):
                        sp = ps_s.tile([128, 1024], f32, tag="sp")
                        nc.tensor.matmul(
                            sp[:, 0:512],
                            kTA[:, 128 * j : 128 * (j + 1)],
                            qs,
                            start=True,
                            stop=True,
                        )
                        nc.tensor.matmul(
                            sp[:, 512:1024],
                            kTB[:, 128 * j : 128 * (j + 1)],
                            qs,
                            start=True,
                            stop=True,
                        )
                        pt = ptp.tile([128, 1024], bf16, tag="pt")
                        nc.scalar.activation(pt[:], sp[:], Exp, scale=2.0 * SCALE)
                        r = j - 4 * t
                        if 0 <= r < 4:  # diagonal block: mask both halves
                            nc.vector.tensor_mul(
                                pt[:], pt[:], maskt[:, 1024 * r : 1024 * (r + 1)]
                            )
                        nc.tensor.matmul(
                            accA[:],
                            voA[:, j, :],
                            pt[:, 0:512],
                            start=(j == 0),
                            stop=(j == nblk - 1),
                        )
                        nc.tensor.matmul(
                            accB[:],
                            voB[:, j, :],
                            pt[:, 512:1024],
                            start=(j == 0),
                            stop=(j == nblk - 1),
                        )
                        if j in (1, 3) and pending:
                            emit_finalize()

                    for h, acc in ((hA, accA), (hB, accB)):
                        acc_sb = finp.tile(
                            [65, 512], f32, tag="acc_sb", name=f"accsb_{h}_{t}"
                        )
                        nc.vector.tensor_copy(acc_sb[:], acc[:])
                        pending.append((h, t, acc_sb))

                while pending:
                    emit_finalize()

    nc.finalize()
    return nc


def _masks_np(dtype_name="bfloat16"):
    import ml_dtypes

    dt = np.float32 if dtype_name.startswith("float32") else ml_dtypes.bfloat16
    j = np.arange(128)[:, None]
    c = np.arange(512)[None, :]
    cols = []
    for r in (0, 128, 256, 384):
        m = (c - j >= r).astype(dt)
        cols.append(m)
        cols.append(m)  # duplicated for the two heads of a pair
    return np.ascontiguousarray(np.concatenate(cols, axis=1))  # [128, 4096]


def get_program(qk_dt="bfloat16", pv_dt="bfloat16"):
    key = (qk_dt, pv_dt)
    if key not in _BUILT:
        _BUILT[key] = _build(qk_dt, pv_dt)
    return _BUILT[key]


def make_in_maps(q, k, v, pv_dt="bfloat16"):
    """Split full [B,H,N,D] inputs into per-core input maps."""
    qf = np.asarray(q, dtype=np.float32).reshape(B * H, N, D)
    kf = np.asarray(k, dtype=np.float32).reshape(B * H, N, D)
    vf = np.asarray(v, dtype=np.float32).reshape(B * H, N, D)
    masks = _masks_np(pv_dt)
    maps = []
    for c in range(NCORES):
        sl = slice(c * HPC, (c + 1) * HPC)
        maps.append(
            {
                "q": np.ascontiguousarray(qf[sl]),
                "k": np.ascontiguousarray(kf[sl]),
                "v": np.ascontiguousarray(vf[sl]),
                "masks": masks,
            }
        )
    return maps


def kernel(q, k, v):
    from concourse.bass_utils import run_bass_kernel_spmd

    nc = get_program()
    maps = make_in_maps(q, k, v)
    res = run_bass_kernel_spmd(nc, maps, list(range(NCORES)))
    out = np.concatenate([res.results[c]["out"] for c in range(NCORES)], axis=0)
    return out.reshape(B, H, N, D)


# revision 20
# speedup vs baseline: 1.1065x; 1.1065x over previous
"""Trainium2 Bass kernel for nn_Attend (l2-distance attention with zero-kv).

Reference computation (per b,h):
    k' = [0; k], v' = [0; v]                       (prepend zero kv)
    scores[i,j] = (2 q_i.k'_j - |q_i|^2 - |k'_j|^2) * (D+2)^-0.5
    causal: j <= i+1 in padded index space
    out = softmax(scores) @ v'

Kernel algebra: softmax is invariant to the per-row constant -scale*|q_i|^2,
so with p~[i,j] = exp(2*scale*q_i.k_j) * exp(-scale*|k_j|^2) and the zero
column contributing exp(0)=1 to the denominator only:
    out_i = (sum_j p~ v_j) / (1 + sum_j p~)

Layout: scores are computed TRANSPOSED ([kv, q]) so P^T is directly the
moving operand of the PV matmul (no P transposes).  exp(-scale*|k|^2) is
folded into the PV stationary operand [V | 1] per kv partition; 2*scale is
folded into the exp activation's free affine scale.

The PE streams the moving operand at half rate when the contraction dim is
<= 64, so heads are processed in PAIRS with K=128: kT2 [128, n] stacks both
heads' k^T; q^T is staged BLOCK-DIAGONALLY (qTp [128, 2n]: head A in rows
0:64 of the first n cols, head B in rows 64:128 of the last n cols, zeros
elsewhere) so one K=128 matmul per head yields that head's scores with the
other head's contribution zeroed.  q^T/k^T are produced without the PE:
gpsimd cast-DMA (fp32->bf16) into DRAM staging, then HWDGE DMA-transpose.

Sharding: 32 (b,h) pairs -> 4 heads per core, 8 cores, pure data parallel.
"""

import sys

for _p in ("/opt/trn_rl_repo", "/root/.axon_site"):
    if _p not in sys.path:
        sys.path.insert(0, _p)

import numpy as np

B, H, N, D = 2, 16, 2048, 64
NCORES = 8
HPC = (B * H) // NCORES          # heads per core = 4
SCALE = float((D + 2) ** -0.5)   # augmented head dim, matches reference
NB = N // 128                    # kv blocks of 128 = 16
NQT = N // 512                   # q tiles of 512 = 4

_BUILT = {}


def _build(qk_dt="bfloat16", pv_dt="bfloat16", hpc=HPC, n=N):
    """Build + finalize the SPMD Bass program (one core's view)."""
    assert qk_dt == "bfloat16" and pv_dt == "bfloat16", "v3 builder is bf16-only"
    assert hpc % 2 == 0, "heads processed in pairs"
    NB = n // 128
    NQT = n // 512
    import concourse.mybir as mybir
    import concourse.tile as tile
    from concourse import bacc
    from concourse.masks import make_identity

    f32 = mybir.dt.float32
    bf16 = mybir.dt.bfloat16
    Exp = mybir.ActivationFunctionType.Exp
    add = mybir.AluOpType.add

    nc = bacc.Bacc("TRN2", target_bir_lowering=False, debug=False, num_swdge_queues=4)
    q_p = nc.declare_dram_parameter("q", [hpc, n, D], f32, isOutput=False)
    k_p = nc.declare_dram_parameter("k", [hpc, n, D], f32, isOutput=False)
    v_p = nc.declare_dram_parameter("v", [hpc, n, D], f32, isOutput=False)
    m_p = nc.declare_dram_parameter("masks", [128, 4 * 1024], bf16, isOutput=False)
    o_p = nc.declare_dram_parameter("out", [hpc, n, D], f32, isOutput=True)

    npairs = hpc // 2

    with tile.TileContext(nc) as tc:
        with (
            tc.tile_pool(name="stg", bufs=2, space="DRAM") as stgp,
            tc.tile_pool(name="const", bufs=1) as constp,
            tc.tile_pool(name="io", bufs=2) as iop,
            tc.tile_pool(name="kqt", bufs=2) as kqtp,
            tc.tile_pool(name="pt", bufs=4) as ptp,
            tc.tile_pool(name="fin", bufs=2) as finp,
            tc.tile_pool(name="vop", bufs=4) as vop,
            tc.tile_pool(name="ps_s", bufs=3, space="PSUM") as ps_s,
            tc.tile_pool(name="ps_af", bufs=2, space="PSUM") as ps_af,
        ):
            ident = constp.tile([128, 128], f32, tag="ident")
            make_identity(nc, ident[:])
            maskt = constp.tile([128, 4 * 1024], bf16, tag="maskt")
            nc.scalar.dma_start(out=maskt[:], in_=m_p[:])

            # ---- staging for ALL pairs up-front ----------------------
            qTps, kT2s = [], []
            for pair in range(npairs):
                hA, hB = 2 * pair, 2 * pair + 1
                stq = stgp.tile([n, 128], bf16, tag="stq")
                stk = stgp.tile([n, 128], bf16, tag="stk")
                nc.gpsimd.dma_start(out=stq[:, 0:64], in_=q_p[hA])
                nc.gpsimd.dma_start(out=stq[:, 64:128], in_=q_p[hB])
                nc.gpsimd.dma_start(out=stk[:, 0:64], in_=k_p[hA])
                nc.gpsimd.dma_start(out=stk[:, 64:128], in_=k_p[hB])
                qT2 = kqtp.tile([128, n], bf16, tag="qT2", name=f"qT2_{pair}")
                kT2 = kqtp.tile([128, n], bf16, tag="kT2", name=f"kT2_{pair}")
                nc.sync.dma_start(out=qT2[:], in_=stq[:], transpose=True)
                nc.sync.dma_start(out=kT2[:], in_=stk[:], transpose=True)
                # block-diagonal qTp assembled on-chip
                qTp = kqtp.tile([128, 2 * n], bf16, tag="qTp", name=f"qTp_{pair}")
                nc.vector.tensor_copy(qTp[0:64, 0:n], qT2[0:64, :])
                nc.vector.memset(qTp[64:128, 0:n], 0.0)
                nc.vector.memset(qTp[0:64, n : 2 * n], 0.0)
                nc.vector.tensor_copy(qTp[64:128, n : 2 * n], qT2[64:128, :])
                qTps.append(qTp)
                kT2s.append(kT2)

            for pair in range(npairs):
                hA, hB = 2 * pair, 2 * pair + 1
                qTp = qTps[pair]
                kT2 = kT2s[pair]

                # ---- per-head: [V*ek | ek] --------------------------
                vos = []
                for h in (hA, hB):
                    kn = iop.tile([128, NB, 64], f32, tag="kn")
                    vn = iop.tile([128, NB, 64], f32, tag="vn")
                    vo = vop.tile([128, NB, 65], bf16, tag="vo")
                    nc.scalar.dma_start(
                        out=kn[:], in_=k_p[h].rearrange("(b p) d -> p b d", p=128)
                    )
                    nc.scalar.dma_start(
                        out=vn[:], in_=v_p[h].rearrange("(b p) d -> p b d", p=128)
                    )
                    scr2 = iop.tile([128, NB, 64], f32, tag="scr2")
                    ksqs = iop.tile([128, NB], f32, tag="ksqs")
                    nc.vector.tensor_mul(scr2[:], kn[:], kn[:])
                    nc.vector.tensor_reduce(
                        ksqs[:], scr2[:], mybir.AxisListType.X, add
                    )
                    ek = iop.tile([128, NB], f32, tag="ek")
                    nc.scalar.activation(ek[:], ksqs[:], Exp, scale=-SCALE)
                    for b in range(NB):
                        nc.vector.tensor_scalar_mul(
                            vo[:, b, 0:64], vn[:, b, :], ek[:, b : b + 1]
                        )
                    nc.vector.tensor_copy(vo[:, :, 64:65], ek[:])
                    vos.append(vo)
                voA, voB = vos

                # ---- main flash loop (both heads per block) ----------
                for t in range(NQT):
                    nblk = 4 * (t + 1)
                    accA = ps_af.tile([65, 512], f32, tag="af", name=f"accA_{pair}_{t}")
                    accB = ps_af.tile([65, 512], f32, tag="af", name=f"accB_{pair}_{t}")
                    qsA = qTp[:, 512 * t : 512 * (t + 1)]
                    qsB = qTp[:, n + 512 * t : n + 512 * (t + 1)]
                    for j in range(nblk):
                        kslc = kT2[:, 128 * j : 128 * (j + 1)]
                        sp = ps_s.tile([128, 1024], f32, tag="sp")
                        nc.tensor.matmul(
                            sp[:, 0:512], kslc, qsA, start=True, stop=True
                        )
                        nc.tensor.matmul(
                            sp[:, 512:1024], kslc, qsB, start=True, stop=True
                        )
                        pt = ptp.tile([128, 1024], bf16, tag="pt")
                        nc.scalar.activation(pt[:], sp[:], Exp, scale=2.0 * SCALE)
                        r = j - 4 * t
                        if 0 <= r < 4:  # diagonal block: mask both halves
                            nc.vector.tensor_mul(
                                pt[:], pt[:], maskt[:, 1024 * r : 1024 * (r + 1)]
                            )
                        nc.tensor.matmul(
                            accA[:],
                            voA[:, j, :],
                            pt[:, 0:512],
                            start=(j == 0),
                            stop=(j == nblk - 1),
                        )
                        nc.tensor.matmul(
                            accB[:],
                            voB[:, j, :],
                            pt[:, 512:1024],
                            start=(j == 0),
                            stop=(j == nblk - 1),
                        )

                    # ---- finalize both heads -------------------------
                    for h, acc in ((hA, accA), (hB, accB)):
                        acc_sb = finp.tile([65, 512], f32, tag="acc_sb")
                        nc.vector.tensor_copy(acc_sb[:], acc[:])
                        ptr4 = ps_s.tile(
                            [128, 4, 65], f32, tag="sp", name=f"ptr4_{pair}_{t}_{h}"
                        )
                        for s in range(4):
                            nc.tensor.matmul(
                                ptr4[:, s, :],
                                acc_sb[:, 128 * s : 128 * (s + 1)],
                                ident[0:65, 0:65],
                                is_transpose=True,
                                start=(s == 0),
                                stop=(s == 3),
                            )
                        outt = finp.tile([128, 4, 64], f32, tag="outt")
                        dr = finp.tile([128, 8], f32, tag="dr")
                        nc.vector.tensor_scalar_add(
                            dr[:, 0:4], ptr4[:, :, 64], 1.0
                        )
                        nc.vector.reciprocal(dr[:, 4:8], dr[:, 0:4])
                        for s in range(4):
                            nc.vector.tensor_scalar_mul(
                                outt[:, s, :],
                                ptr4[:, s, 0:64],
                                dr[:, 4 + s : 5 + s],
                            )
                        nc.scalar.dma_start(
                            out=o_p[h].rearrange("(s p) d -> p s d", p=128)[
                                :, 4 * t : 4 * (t + 1), :
                            ],
                            in_=outt[:],
                        )

    nc.finalize()
    return nc


def _masks_np(dtype_name="bfloat16"):
    import ml_dtypes

    dt = np.float32 if dtype_name.startswith("float32") else ml_dtypes.bfloat16
    j = np.arange(128)[:, None]
    c = np.arange(512)[None, :]
    cols = []
    for r in (0, 128, 256, 384):
        m = (c - j >= r).astype(dt)
        cols.append(m)
        cols.append(m)  # duplicated for the two heads of a pair
    return np.ascontiguousarray(np.concatenate(cols, axis=1))  # [128, 4096]


def get_program(qk_dt="bfloat16", pv_dt="bfloat16"):
    key = (qk_dt, pv_dt)
    if key not in _BUILT:
        _BUILT[key] = _build(qk_dt, pv_dt)
    return _BUILT[key]


def make_in_maps(q, k, v, pv_dt="bfloat16"):
    """Split full [B,H,N,D] inputs into per-core input maps."""
    qf = np.asarray(q, dtype=np.float32).reshape(B * H, N, D)
    kf = np.asarray(k, dtype=np.float32).reshape(B * H, N, D)
    vf = np.asarray(v, dtype=np.float32).reshape(B * H, N, D)
    masks = _masks_np(pv_dt)
    maps = []
    for c in range(NCORES):
        sl = slice(c * HPC, (c + 1) * HPC)
        maps.append(
            {
                "q": np.ascontiguousarray(qf[sl]),
                "k": np.ascontiguousarray(kf[sl]),
                "v": np.ascontiguousarray(vf[sl]),
                "masks": masks,
            }
        )
    return maps


def kernel(q, k, v):
    from concourse.bass_utils import run_bass_kernel_spmd

    nc = get_program()
    maps = make_in_maps(q, k, v)
    res = run_bass_kernel_spmd(nc, maps, list(range(NCORES)))
    out = np.concatenate([res.results[c]["out"] for c in range(NCORES)], axis=0)
    return out.reshape(B, H, N, D)
